# revision 22
# baseline (speedup 1.0000x reference)
"""Trainium2 Bass kernel for ConstrainedMLP (B=262144, 12->256->256->12 MLP + constraints).

Data-parallel across 8 NeuronCores: batch split 8 x 32768, tiny weights replicated.

Per-core pipeline (feature-major matmuls, batch-major epilogue):
  - x loaded batch-major [128, c, 12]; hi/lo fp16 split built batch-major, then
    PE-transposed to feature-major (one K=36 stacked transpose per 128-chunk)
  - L1 as ONE matmul per m-tile with K=36 stacked operands
    ([W1hi; W1lo; W1hi] . [xhi; xhi; xlo]) - 3-term compensated fp16 product
  - L2/L3 as 3-term fp16 matmuls (Whi.hhi + Wlo.hhi + Whi.hlo), fp32 PSUM accum
  - relu+bias fused into the PSUM->SBUF copy on the scalar engine; hi/lo splits
    of activations on gpsimd (h1) and vector (h2)
  - y PE-transposed back to batch-major; constraint epilogue on DVE/ACT/Pool
    with Newton-iteration rsqrt (no ACT Sqrt -> single activation table)
"""
import sys

sys.path.insert(0, "/opt/trn_rl_repo")

from contextlib import ExitStack

import numpy as np

import concourse.bass as bass
import concourse.tile as tile
from concourse import bacc, mybir
from concourse.bass_utils import run_bass_kernel_spmd
from concourse.masks import make_identity

B, IN, HID, OUT = 262144, 12, 256, 12
NCORES = 8
BC = B // NCORES          # 32768 batch rows per core
SUP = 512                 # batch rows per supertile
NCH = SUP // 128          # 4 chunks of 128 rows
NSUP = BC // SUP          # 64 supertiles
F32 = mybir.dt.float32
F16 = mybir.dt.float16
ALU = mybir.AluOpType
ACTF = mybir.ActivationFunctionType
AX = mybir.AxisListType

MAGIC = 0x5F3759DF  # rsqrt seed


def _rsqrt(nc, eng, pool, a, shape, tag):
    """Newton rsqrt of fp32 tile `a` (3 iterations, ~1e-9 rel). Returns tile."""
    y = pool.tile(shape, F32, tag=f"{tag}_y", name=f"{tag}_y")
    t = pool.tile(shape, F32, tag=f"{tag}_t", name=f"{tag}_t")
    yi = y[:].bitcast(mybir.dt.int32)
    # seed: y = bits(MAGIC - (bits(a) >> 1))
    eng.tensor_single_scalar(yi, a[:].bitcast(mybir.dt.int32), 1,
                             ALU.arith_shift_right)
    eng.tensor_scalar(yi, yi, MAGIC, -1, ALU.subtract, ALU.mult)
    for _ in range(3):
        eng.tensor_tensor(t[:], y[:], y[:], ALU.mult)                   # y*y
        eng.scalar_tensor_tensor(t[:], t[:], -0.5, a[:], ALU.mult, ALU.mult)
        eng.scalar_tensor_tensor(y[:], t[:], 1.5, y[:], ALU.add, ALU.mult)
    return y


def _build(nsup=NSUP, debug_raw_y=False):
    bc = nsup * SUP
    nc = bacc.Bacc(None)
    inp_h = nc.declare_dram_parameter("inp", [bc, IN], F32, isOutput=False)
    W1_h = nc.declare_dram_parameter("W1", [IN, HID], F32, isOutput=False)
    b1_h = nc.declare_dram_parameter("b1", [HID], F32, isOutput=False)
    W2_h = nc.declare_dram_parameter("W2", [HID, HID], F32, isOutput=False)
    b2_h = nc.declare_dram_parameter("b2", [HID], F32, isOutput=False)
    W3_h = nc.declare_dram_parameter("W3", [HID, OUT], F32, isOutput=False)
    b3_h = nc.declare_dram_parameter("b3", [OUT], F32, isOutput=False)
    out_h = nc.declare_dram_parameter("out", [bc, IN], F32, isOutput=True)

    with tile.TileContext(nc) as tc, ExitStack() as ctx:
        const = ctx.enter_context(tc.tile_pool(name="const", bufs=1))
        xpool = ctx.enter_context(tc.tile_pool(name="xp", bufs=4))
        spool = ctx.enter_context(tc.tile_pool(name="sp", bufs=3))
        opool = ctx.enter_context(tc.tile_pool(name="op", bufs=4))
        pps = ctx.enter_context(tc.tile_pool(name="pps", bufs=1, space="PSUM"))
        pps2 = ctx.enter_context(tc.tile_pool(name="pps2", bufs=1, space="PSUM"))

        # ---- constants (loaded once) ----
        w1f = const.tile([IN, HID], F32)
        nc.sync.dma_start(out=w1f[:], in_=W1_h[:])
        w1hi = const.tile([IN, HID], F16)
        nc.vector.tensor_copy(w1hi[:], w1f[:])
        w1lo = const.tile([IN, HID], F16)
        nc.vector.tensor_tensor(w1lo[:], w1f[:], w1hi[:], ALU.subtract)
        # K-stacked L1 weights: rows [0:12]=W1hi, [12:24]=W1lo, [24:36]=W1hi
        W1st = const.tile([3 * IN, HID], F16)
        nc.vector.tensor_copy(W1st[0:IN, :], w1hi[:])
        nc.sync.dma_start(out=W1st[IN:2 * IN, :], in_=w1lo[:])      # partition shift
        nc.sync.dma_start(out=W1st[2 * IN:3 * IN, :], in_=w1hi[:])  # partition shift

        w2f = const.tile([128, 2, HID], F32)
        nc.sync.dma_start(out=w2f[:], in_=W2_h[:].rearrange("(kt p) j -> p kt j", p=128))
        W2hi = const.tile([128, 2, HID], F16)
        nc.vector.tensor_copy(W2hi[:], w2f[:])
        W2lo = const.tile([128, 2, HID], F16)
        nc.vector.tensor_tensor(W2lo[:], w2f[:], W2hi[:], ALU.subtract)

        w3f = const.tile([128, 2, OUT], F32)
        nc.sync.dma_start(out=w3f[:], in_=W3_h[:].rearrange("(kt p) j -> p kt j", p=128))
        W3hi = const.tile([128, 2, OUT], F16)
        nc.vector.tensor_copy(W3hi[:], w3f[:])
        W3lo = const.tile([128, 2, OUT], F16)
        nc.vector.tensor_tensor(W3lo[:], w3f[:], W3hi[:], ALU.subtract)

        b1sb = const.tile([128, 2], F32)
        nc.sync.dma_start(out=b1sb[:], in_=b1_h[:].rearrange("(m p) -> p m", p=128))
        b2sb = const.tile([128, 2], F32)
        nc.sync.dma_start(out=b2sb[:], in_=b2_h[:].rearrange("(m p) -> p m", p=128))
        b3sb = const.tile([OUT, 1], F32)
        nc.sync.dma_start(out=b3sb[:], in_=b3_h[:].unsqueeze(1))

        id128h = const.tile([128, 128], F16)
        make_identity(nc, id128h[:])
        id12 = const.tile([OUT, OUT], F32)
        make_identity(nc, id12[:])

        inp_r = inp_h[:].rearrange("(s c p) f -> s p c f", p=128, c=NCH)
        out_r = out_h[:].rearrange("(s c p) f -> s p c f", p=128, c=NCH)

        for s in range(nsup):
            # ---- load x; build K-stacked fp16 [xhi; xhi; xlo] batch-major ----
            x = xpool.tile([128, NCH, IN], F32, tag="x")
            nc.sync.dma_start(out=x[:], in_=inp_r[s])
            x36 = xpool.tile([128, NCH, 3 * IN], F16, tag="x36")
            nc.vector.tensor_copy(x36[:, :, 0:IN], x[:])
            nc.vector.tensor_copy(x36[:, :, IN:2 * IN], x[:])
            nc.vector.tensor_tensor(x36[:, :, 2 * IN:3 * IN], x[:],
                                    x36[:, :, 0:IN], ALU.subtract)

            # ---- transpose: [128, 36] -> [36, 128] per chunk ----
            xT_ps = pps.tile([3 * IN, NCH, 128], F16, tag="xT")
            for c in range(NCH):
                nc.tensor.transpose(xT_ps[:, c], x36[:, c], id128h[:])
            xT = spool.tile([3 * IN, NCH * 128], F16, tag="xTs")
            nc.vector.tensor_copy(xT[:], xT_ps[:].rearrange("f c p -> f (c p)"))

            # ---- L1: one stacked matmul per m-tile ----
            h1_ps = [pps.tile([128, SUP], F32, tag=f"h1_{m}", name=f"h1ps{m}")
                     for m in range(2)]
            for m in range(2):
                nc.tensor.matmul(h1_ps[m][:], W1st[:, m * 128:(m + 1) * 128], xT[:],
                                 start=True, stop=True)
            h1f = spool.tile([128, 2, SUP], F32, tag="h1f")
            for m in range(2):
                nc.scalar.activation(h1f[:, m], h1_ps[m][:], ACTF.Relu,
                                     bias=b1sb[:, m:m + 1], scale=1.0)
            h1hi = spool.tile([128, 2, SUP], F16, tag="h1hi")
            nc.gpsimd.tensor_copy(h1hi[:], h1f[:])
            h1lo = spool.tile([128, 2, SUP], F16, tag="h1lo")
            nc.gpsimd.tensor_tensor(h1lo[:], h1f[:], h1hi[:], ALU.subtract)

            # ---- L2: 3-term fp16 per (m, k) ----
            h2_ps = [pps.tile([128, SUP], F32, tag=f"h2_{m}", name=f"h2ps{m}")
                     for m in range(2)]
            for m in range(2):
                ms = slice(m * 128, (m + 1) * 128)
                nc.tensor.matmul(h2_ps[m][:], W2hi[:, 0, ms], h1hi[:, 0],
                                 start=True, stop=False)
                nc.tensor.matmul(h2_ps[m][:], W2lo[:, 0, ms], h1hi[:, 0],
                                 start=False, stop=False)
                nc.tensor.matmul(h2_ps[m][:], W2hi[:, 0, ms], h1lo[:, 0],
                                 start=False, stop=False)
                nc.tensor.matmul(h2_ps[m][:], W2hi[:, 1, ms], h1hi[:, 1],
                                 start=False, stop=False)
                nc.tensor.matmul(h2_ps[m][:], W2lo[:, 1, ms], h1hi[:, 1],
                                 start=False, stop=False)
                nc.tensor.matmul(h2_ps[m][:], W2hi[:, 1, ms], h1lo[:, 1],
                                 start=False, stop=True)
            h2f = spool.tile([128, 2, SUP], F32, tag="h2f")
            for m in range(2):
                nc.scalar.activation(h2f[:, m], h2_ps[m][:], ACTF.Relu,
                                     bias=b2sb[:, m:m + 1], scale=1.0)
            h2hi = spool.tile([128, 2, SUP], F16, tag="h2hi")
            nc.vector.tensor_copy(h2hi[:], h2f[:])
            h2lo = spool.tile([128, 2, SUP], F16, tag="h2lo")
            nc.vector.tensor_tensor(h2lo[:], h2f[:], h2hi[:], ALU.subtract)

            # ---- L3: 3-term fp16, yT [12, SUP] ----
            yT_ps = pps2.tile([OUT, SUP], F32, tag="yT")
            nc.tensor.matmul(yT_ps[:], W3hi[:, 0], h2hi[:, 0], start=True, stop=False)
            nc.tensor.matmul(yT_ps[:], W3lo[:, 0], h2hi[:, 0], start=False, stop=False)
            nc.tensor.matmul(yT_ps[:], W3hi[:, 0], h2lo[:, 0], start=False, stop=False)
            nc.tensor.matmul(yT_ps[:], W3hi[:, 1], h2hi[:, 1], start=False, stop=False)
            nc.tensor.matmul(yT_ps[:], W3lo[:, 1], h2hi[:, 1], start=False, stop=False)
            nc.tensor.matmul(yT_ps[:], W3hi[:, 1], h2lo[:, 1], start=False, stop=True)
            yTb = spool.tile([OUT, SUP], F32, tag="yTb")
            nc.scalar.activation(yTb[:], yT_ps[:], ACTF.Identity,
                                 bias=b3sb[:, 0:1], scale=1.0)

            # ---- transpose back: y [128, c, 12] ----
            y_ps = pps2.tile([128, NCH, OUT], F32, tag="y", bufs=2)
            for c in range(NCH):
                nc.tensor.transpose(y_ps[:, c], yTb[:, c * 128:(c + 1) * 128], id12[:])

            # ================= epilogue (batch-major) =================
            if debug_raw_y:
                oy = opool.tile([128, NCH, IN], F32, tag="oy")
                nc.vector.tensor_copy(oy[:], y_ps[:])
                nc.sync.dma_start(out=out_r[s], in_=oy[:])
                continue
            o = opool.tile([128, NCH, IN], F32, tag="o")
            nc.scalar.activation(o[:], y_ps[:], ACTF.Tanh)
            # pts = max(sigmoid(y6), prev)
            sg = opool.tile([128, NCH, 1], F32, tag="sg")
            nc.scalar.activation(sg[:], y_ps[:, :, 6:7], ACTF.Sigmoid)
            nc.vector.tensor_tensor(o[:, :, 6:7], sg[:], x[:, :, 6:7], ALU.max)

            # norm^2 batch: [:, :, 0]=pos d2, 1=deputy dn2, 2=sun n2
            nsq = opool.tile([128, NCH, 3], F32, tag="nsq")
            t3 = opool.tile([128, NCH, 3], F32, tag="t3")
            nc.vector.tensor_tensor(t3[:], o[:, :, 0:3], o[:, :, 0:3], ALU.mult)
            nc.vector.tensor_reduce(nsq[:, :, 0:1], t3[:], AX.X, ALU.add)
            dp = opool.tile([128, NCH, 3], F32, tag="dp")
            nc.gpsimd.tensor_tensor(dp[:], x[:, :, 0:3], x[:, :, 7:10], ALU.subtract)
            t3b = opool.tile([128, NCH, 3], F32, tag="t3b")
            nc.gpsimd.tensor_tensor(t3b[:], dp[:], dp[:], ALU.mult)
            nc.vector.tensor_reduce(nsq[:, :, 1:2], t3b[:], AX.X, ALU.add)
            t2 = opool.tile([128, NCH, 2], F32, tag="t2")
            nc.gpsimd.tensor_tensor(t2[:], o[:, :, 10:12], o[:, :, 10:12], ALU.mult)
            nc.vector.tensor_reduce(nsq[:, :, 2:3], t2[:], AX.X, ALU.add)
            rsq = _rsqrt(nc, nc.vector, opool, nsq, [128, NCH, 3], "rsA")

            # --- pos: scale = min(rsqrt(d2), 1) ---
            r1 = opool.tile([128, NCH], F32, tag="r1")
            nc.vector.tensor_scalar(r1[:], rsq[:, :, 0], 1.0, None, ALU.min)
            nc.vector.tensor_tensor(
                o[:, :, 0:3], o[:, :, 0:3],
                r1[:].unsqueeze(2).broadcast_to([128, NCH, 3]), ALU.mult)

            # --- clus ---
            d3 = opool.tile([128, NCH, 3], F32, tag="d3")
            nc.gpsimd.tensor_tensor(d3[:], o[:, :, 7:10], x[:, :, 7:10], ALU.subtract)
            nc.gpsimd.tensor_tensor(t3b[:], d3[:], dp[:], ALU.mult)
            dd = opool.tile([128, NCH], F32, tag="dd")
            nc.vector.tensor_reduce(dd[:], t3b[:], AX.X, ALU.add)
            # w = (dd > 0) * rsqrt(dn2); offset = clus - w * deputy
            msk = opool.tile([128, NCH], F32, tag="msk")
            nc.vector.tensor_single_scalar(msk[:], dd[:], 0.0, ALU.is_gt)
            nc.vector.tensor_tensor(msk[:], msk[:], rsq[:, :, 1], ALU.mult)
            off = opool.tile([128, NCH, 3], F32, tag="off")
            nc.vector.tensor_tensor(
                off[:], dp[:], msk[:].unsqueeze(2).broadcast_to([128, NCH, 3]), ALU.mult)
            nc.vector.tensor_tensor(off[:], o[:, :, 7:10], off[:], ALU.subtract)
            # cd2 and select
            nc.gpsimd.tensor_tensor(t3b[:], off[:], off[:], ALU.mult)
            cd2 = opool.tile([128, NCH], F32, tag="cd2")
            nc.vector.tensor_reduce(cd2[:], t3b[:], AX.X, ALU.add)
            cm = opool.tile([128, NCH], F32, tag="cm")
            nc.gpsimd.tensor_single_scalar(cm[:], cd2[:], 1.0, ALU.is_gt)
            rsB = _rsqrt(nc, nc.vector, opool, cd2, [128, NCH], "rsB")
            nc.vector.tensor_tensor(
                off[:], off[:], rsB[:].unsqueeze(2).broadcast_to([128, NCH, 3]), ALU.mult)
            # blend: clus + (cd2>1) * (off*rsqrt - clus)
            nc.vector.tensor_tensor(off[:], off[:], o[:, :, 7:10], ALU.subtract)
            nc.vector.tensor_tensor(
                off[:], off[:], cm[:].unsqueeze(2).broadcast_to([128, NCH, 3]), ALU.mult)
            nc.vector.tensor_tensor(o[:, :, 7:10], o[:, :, 7:10], off[:], ALU.add)

            # --- sun ---
            nc.vector.tensor_tensor(
                o[:, :, 10:12], o[:, :, 10:12],
                rsq[:, :, 2:3].broadcast_to([128, NCH, 2]), ALU.mult)

            # ---- store ----
            nc.sync.dma_start(out=out_r[s], in_=o[:])

    nc.finalize()
    return nc


_CACHED_NC = None


def kernel(**inputs: np.ndarray) -> np.ndarray:
    global _CACHED_NC
    if _CACHED_NC is None:
        _CACHED_NC = _build()
    nc = _CACHED_NC
    inp = np.ascontiguousarray(inputs["inp"], dtype=np.float32)
    shared = {k: np.ascontiguousarray(inputs[k], dtype=np.float32)
              for k in ("W1", "b1", "W2", "b2", "W3", "b3")}
    in_maps = [dict(shared, inp=inp[i * BC:(i + 1) * BC]) for i in range(NCORES)]
    res = run_bass_kernel_spmd(nc, in_maps, list(range(NCORES)))
    return np.concatenate([res.results[i]["out"] for i in range(NCORES)], axis=0)


# revision 25
# speedup vs baseline: 1.0060x; 1.0060x over previous
"""Trainium2 Bass kernel for ConstrainedMLP (B=262144, 12->256->256->12 MLP + constraints).

Data-parallel across 8 NeuronCores: batch is split 8 x 32768, tiny weights
replicated. Per core:
  - load x in natural [128, c, 12] batch-major tiles
  - PE-transpose 128x12 chunks -> feature-major xT [12, 512]
  - L1/L2/L3 as float32r matmuls (1 cycle/column at N=512) with weights stationary
  - relu+bias fused into the PSUM->SBUF copy on the scalar engine
  - PE-transpose y back to batch-major, run the constraint epilogue on
    vector/scalar/gpsimd engines, DMA out
"""
import sys

sys.path.insert(0, "/opt/trn_rl_repo")

from contextlib import ExitStack

import numpy as np

import concourse.bass as bass
import concourse.tile as tile
from concourse import bacc, mybir
from concourse.bass_utils import run_bass_kernel_spmd
from concourse.masks import make_identity

B, IN, HID, OUT = 262144, 12, 256, 12
NCORES = 8
BC = B // NCORES          # 32768 batch rows per core
SUP = 512                 # batch rows per supertile
NCH = SUP // 128          # 4 chunks of 128 rows
NSUP = BC // SUP          # 64 supertiles
F32 = mybir.dt.float32
F32R = mybir.dt.float32r
ALU = mybir.AluOpType
ACTF = mybir.ActivationFunctionType
AX = mybir.AxisListType


def _build(nsup=NSUP, debug_raw_y=False):
    bc = nsup * SUP
    nc = bacc.Bacc(None)
    inp_h = nc.declare_dram_parameter("inp", [bc, IN], F32, isOutput=False)
    W1_h = nc.declare_dram_parameter("W1", [IN, HID], F32, isOutput=False)
    b1_h = nc.declare_dram_parameter("b1", [HID], F32, isOutput=False)
    W2_h = nc.declare_dram_parameter("W2", [HID, HID], F32, isOutput=False)
    b2_h = nc.declare_dram_parameter("b2", [HID], F32, isOutput=False)
    W3_h = nc.declare_dram_parameter("W3", [HID, OUT], F32, isOutput=False)
    b3_h = nc.declare_dram_parameter("b3", [OUT], F32, isOutput=False)
    out_h = nc.declare_dram_parameter("out", [bc, IN], F32, isOutput=True)

    with tile.TileContext(nc) as tc, ExitStack() as ctx:
        const = ctx.enter_context(tc.tile_pool(name="const", bufs=1))
        xpool = ctx.enter_context(tc.tile_pool(name="xp", bufs=6))
        spool = ctx.enter_context(tc.tile_pool(name="sp", bufs=4))
        opool = ctx.enter_context(tc.tile_pool(name="op", bufs=6))
        pps = ctx.enter_context(tc.tile_pool(name="pps", bufs=1, space="PSUM"))
        pps2 = ctx.enter_context(tc.tile_pool(name="pps2", bufs=1, space="PSUM"))

        # ---- constants (loaded once) ----
        W1sb = const.tile([IN, HID], F32)
        nc.sync.dma_start(out=W1sb[:], in_=W1_h[:])
        W2sb = const.tile([128, 2, HID], F32)
        nc.sync.dma_start(out=W2sb[:], in_=W2_h[:].rearrange("(kt p) j -> p kt j", p=128))
        W3sb = const.tile([128, 2, OUT], F32)
        nc.sync.dma_start(out=W3sb[:], in_=W3_h[:].rearrange("(kt p) j -> p kt j", p=128))

        b1sb = const.tile([128, 2], F32)
        nc.sync.dma_start(out=b1sb[:], in_=b1_h[:].rearrange("(m p) -> p m", p=128))
        b2sb = const.tile([128, 2], F32)
        nc.sync.dma_start(out=b2sb[:], in_=b2_h[:].rearrange("(m p) -> p m", p=128))
        b3sb = const.tile([OUT, 1], F32)
        nc.sync.dma_start(out=b3sb[:], in_=b3_h[:].unsqueeze(1))

        id128f = const.tile([128, 128], F32)
        make_identity(nc, id128f[:])
        id128r = id128f[:]
        id12 = const.tile([OUT, OUT], F32)
        make_identity(nc, id12[:])

        inp_r = inp_h[:].rearrange("(s c p) f -> s p c f", p=128, c=NCH)
        out_r = out_h[:].rearrange("(s c p) f -> s p c f", p=128, c=NCH)

        for s in range(nsup):
            # ---- load x ----
            x = xpool.tile([128, NCH, IN], F32, tag="x")
            nc.sync.dma_start(out=x[:], in_=inp_r[s])
            xr = xpool.tile([128, NCH, IN], F32, tag="xr")
            nc.vector.tensor_copy(xr[:], x[:])

            # ---- transpose x chunks: [128,12] -> [12,128] ----
            xT_ps = pps.tile([IN, NCH, 128], F32, tag="xT")
            for c in range(NCH):
                nc.tensor.transpose(xT_ps[:, c], xr[:, c], id128r)
            xT = spool.tile([IN, NCH * 128], F32, tag="xTs")
            nc.vector.tensor_copy(xT[:], xT_ps[:].rearrange("f c p -> f (c p)"))

            # ---- L1: h1T[m*128+j, n] ----
            h1_ps = [pps.tile([128, SUP], F32, tag=f"h1_{m}", name=f"h1ps{m}")
                     for m in range(2)]
            for m in range(2):
                nc.tensor.matmul(h1_ps[m][:], W1sb[:, m * 128:(m + 1) * 128], xT[:],
                                 start=True, stop=True)
            h1T = spool.tile([128, 2, SUP], F32, tag="h1T")
            for m in range(2):
                nc.scalar.activation(h1T[:, m], h1_ps[m][:], ACTF.Relu,
                                     bias=b1sb[:, m:m + 1], scale=1.0)

            # ---- L2 ----
            h2_ps = [pps.tile([128, SUP], F32, tag=f"h2_{m}", name=f"h2ps{m}")
                     for m in range(2)]
            for m in range(2):
                nc.tensor.matmul(h2_ps[m][:], W2sb[:, 0, m * 128:(m + 1) * 128],
                                 h1T[:, 0], start=True, stop=False)
                nc.tensor.matmul(h2_ps[m][:], W2sb[:, 1, m * 128:(m + 1) * 128],
                                 h1T[:, 1], start=False, stop=True)
            h2T = spool.tile([128, 2, SUP], F32, tag="h2T")
            for m in range(2):
                nc.scalar.activation(h2T[:, m], h2_ps[m][:], ACTF.Relu,
                                     bias=b2sb[:, m:m + 1], scale=1.0)

            # ---- L3: yT [12, SUP] ----
            yT_ps = pps2.tile([OUT, SUP], F32, tag="yT")
            nc.tensor.matmul(yT_ps[:], W3sb[:, 0], h2T[:, 0], start=True, stop=False)
            nc.tensor.matmul(yT_ps[:], W3sb[:, 1], h2T[:, 1], start=False, stop=True)
            yTb = spool.tile([OUT, SUP], F32, tag="yTb")
            nc.scalar.activation(yTb[:], yT_ps[:], ACTF.Identity,
                                 bias=b3sb[:, 0:1], scale=1.0)

            # ---- transpose back: y [128, c, 12] ----
            y_ps = pps2.tile([128, NCH, OUT], F32, tag="y", bufs=2)
            for c in range(NCH):
                nc.tensor.transpose(y_ps[:, c], yTb[:, c * 128:(c + 1) * 128], id12[:])

            # ================= epilogue (batch-major) =================
            if debug_raw_y:
                oy = opool.tile([128, NCH, IN], F32, tag="oy")
                nc.vector.tensor_copy(oy[:], y_ps[:])
                nc.sync.dma_start(out=out_r[s], in_=oy[:])
                continue
            o = opool.tile([128, NCH, IN], F32, tag="o")
            # tanh of everything (col 6 fixed below)
            nc.scalar.activation(o[:], y_ps[:], ACTF.Tanh)
            # pts = max(sigmoid(y6), prev)
            sg = opool.tile([128, NCH, 1], F32, tag="sg")
            nc.scalar.activation(sg[:], y_ps[:, :, 6:7], ACTF.Sigmoid)
            nc.vector.tensor_tensor(o[:, :, 6:7], sg[:], x[:, :, 6:7], ALU.max)

            # --- pos: clip to unit sphere (scale = min(1/dist, 1)) ---
            t3 = opool.tile([128, NCH, 3], F32, tag="t3")
            nc.vector.tensor_tensor(t3[:], o[:, :, 0:3], o[:, :, 0:3], ALU.mult)
            r1 = opool.tile([128, NCH], F32, tag="r1")
            nc.vector.tensor_reduce(r1[:], t3[:], AX.X, ALU.add)
            nc.scalar.activation(r1[:], r1[:], ACTF.Sqrt)        # dist
            nc.vector.reciprocal(r1[:], r1[:])                   # 1/dist
            nc.vector.tensor_scalar(r1[:], r1[:], 1.0, None, ALU.min)
            nc.vector.tensor_tensor(
                o[:, :, 0:3], o[:, :, 0:3],
                r1[:].unsqueeze(2).broadcast_to([128, NCH, 3]), ALU.mult)

            # --- clus ---
            dp = opool.tile([128, NCH, 3], F32, tag="dp")
            nc.gpsimd.tensor_tensor(dp[:], x[:, :, 0:3], x[:, :, 7:10], ALU.subtract)
            d3 = opool.tile([128, NCH, 3], F32, tag="d3")
            nc.gpsimd.tensor_tensor(d3[:], o[:, :, 7:10], x[:, :, 7:10], ALU.subtract)
            # dd = <delta, deputy>
            t3b = opool.tile([128, NCH, 3], F32, tag="t3b")
            nc.gpsimd.tensor_tensor(t3b[:], d3[:], dp[:], ALU.mult)
            dd = opool.tile([128, NCH], F32, tag="dd")
            nc.vector.tensor_reduce(dd[:], t3b[:], AX.X, ALU.add)
            # dnorm
            nc.vector.tensor_tensor(t3b[:], dp[:], dp[:], ALU.mult)
            dn = opool.tile([128, NCH], F32, tag="dn")
            nc.vector.tensor_reduce(dn[:], t3b[:], AX.X, ALU.add)
            nc.scalar.activation(dn[:], dn[:], ACTF.Sqrt)
            nc.vector.reciprocal(dn[:], dn[:])                   # 1/|deputy|
            # w = (dd > 0) * (1/|deputy|)   -> offset = clus - w * deputy
            msk = opool.tile([128, NCH], F32, tag="msk")
            nc.vector.tensor_single_scalar(msk[:], dd[:], 0.0, ALU.is_gt)
            nc.vector.tensor_tensor(dn[:], dn[:], msk[:], ALU.mult)
            off = opool.tile([128, NCH, 3], F32, tag="off")
            nc.vector.tensor_tensor(
                off[:], dp[:], dn[:].unsqueeze(2).broadcast_to([128, NCH, 3]), ALU.mult)
            nc.vector.tensor_tensor(off[:], o[:, :, 7:10], off[:], ALU.subtract)
            # cdist, select
            nc.gpsimd.tensor_tensor(t3b[:], off[:], off[:], ALU.mult)
            cd = opool.tile([128, NCH], F32, tag="cd")
            nc.vector.tensor_reduce(cd[:], t3b[:], AX.X, ALU.add)
            nc.scalar.activation(cd[:], cd[:], ACTF.Sqrt)
            cm = opool.tile([128, NCH], F32, tag="cm")
            nc.gpsimd.tensor_single_scalar(cm[:], cd[:], 1.0, ALU.is_gt)
            nc.vector.reciprocal(cd[:], cd[:])
            nc.vector.tensor_tensor(
                off[:], off[:], cd[:].unsqueeze(2).broadcast_to([128, NCH, 3]), ALU.mult)
            # blend: clus + (cdist>1) * (off/cdist - clus)
            nc.vector.tensor_tensor(off[:], off[:], o[:, :, 7:10], ALU.subtract)
            nc.vector.tensor_tensor(
                off[:], off[:], cm[:].unsqueeze(2).broadcast_to([128, NCH, 3]), ALU.mult)
            nc.vector.tensor_tensor(o[:, :, 7:10], o[:, :, 7:10], off[:], ALU.add)

            # --- sun: project to unit circle ---
            t2 = opool.tile([128, NCH, 2], F32, tag="t2")
            nc.gpsimd.tensor_tensor(t2[:], o[:, :, 10:12], o[:, :, 10:12], ALU.mult)
            sn = opool.tile([128, NCH], F32, tag="sn")
            nc.vector.tensor_reduce(sn[:], t2[:], AX.X, ALU.add)
            nc.scalar.activation(sn[:], sn[:], ACTF.Sqrt)
            nc.vector.reciprocal(sn[:], sn[:])
            nc.vector.tensor_tensor(
                o[:, :, 10:12], o[:, :, 10:12],
                sn[:].unsqueeze(2).broadcast_to([128, NCH, 2]), ALU.mult)

            # ---- store ----
            nc.sync.dma_start(out=out_r[s], in_=o[:])

    nc.finalize()
    return nc


_CACHED_NC = None


def kernel(**inputs: np.ndarray) -> np.ndarray:
    global _CACHED_NC
    if _CACHED_NC is None:
        _CACHED_NC = _build()
    nc = _CACHED_NC
    inp = np.ascontiguousarray(inputs["inp"], dtype=np.float32)
    shared = {k: np.ascontiguousarray(inputs[k], dtype=np.float32)
              for k in ("W1", "b1", "W2", "b2", "W3", "b3")}
    in_maps = [dict(shared, inp=inp[i * BC:(i + 1) * BC]) for i in range(NCORES)]
    res = run_bass_kernel_spmd(nc, in_maps, list(range(NCORES)))
    return np.concatenate([res.results[i]["out"] for i in range(NCORES)], axis=0)
